# revision 46
# baseline (speedup 1.0000x reference)
"""Trainium2 Bass kernel for AxisLengthNetMetric (chamfer-distance + L1-size metric).

Reference computation (per row n of N = 262144):
  gt_box row -> size s (cols 3:6), rx (6:9), ry (9:12)
  rx_hat = rx/|rx|, ry_hat = ry/|ry|, rz = cross(rx_hat, ry_hat)   (rz NOT normalized)
  corners u_c = sum_k sign[c,k] * 0.5*s[k] * axis_k   (8 corners, +-pairs)
  chamfer(corners, pred_pts[n]): d[c,q] = |u_c - b_q|^2, dist1 = min_q, dist2 = min_c
  out[0] = mean over (N,8) of dist1+dist2 ; out[1] = mean |s - pred_size|

Kernel strategy (v6):
- data parallel over 8 cores; per core 32768 rows as 128 partitions x 256.
- host casts pred/pred_size/gt(9 used cols) to bf16 (halves DMA; error still ~6e-5).
- v' = [c0*rx, c1*ry, cz*(rx x ry)] = 2*[v0,v1,v2]; dot products h[k,q] = v'_k.b_q
  as ONE bf16 2x-mode TensorTensor (d innermost; broadcasts only on outer dims,
  which preserves the 2x DVE mode) + d-sum adds; u' = 2u is never materialized.
- with w01 = [h0+h1, h0-h1] and t_pm = b2 -+ h2, all 8 em/ep rows are single
  adds/subs of broadcast-patterned (w01, t_pm) pairs -> shared (g,8,8) tile whose
  q-halving min tree gives dist1; the final tree level and the dist2 values share
  one (g,16) tile so a single ACT identity-accumulate yields the min-sums.
- dist2 = b2 + min_c(a2_c - |g'|) where a2 has only TWO values D+- = |v0+-v1|^2
  + |v2|^2 whose parity matches the corner signs, and absmax(g'_0,g'_2) =
  |w01p| + |h2| (max(|a+b|,|a-b|) = |a|+|b|), so dist2 needs no g' tensor at all:
  dist2_q = min(D+ - |w01p|, D- - |w01m|) - |h2|.
- per-row sums accumulate for free via ACT accum_out (squares -> SQB, D-scalars
  -> SQA, mins -> E3S, |size diff| -> L1S); host: cd_sum = E3 + 8*sum(A) + sum(b2).
- phase 0 runs as 2 chunks, each split into a critical head (axes -> vta) and a
  deferrable tail (D scalars) so chunk-1's head is not queued behind chunk-0's
  tail on the in-order engine sequencers; phase 1 is emitted
  software-pipelined in wavefront order (stage s of tile t at wave s+t) because
  engine sequencers are in-order with shallow lookahead -- tile-major emission
  serializes the whole kernel on cross-engine dependency chains.
- engine split respects TRN2 ISA legality: Pool only ever runs add/sub/mul
  (no min/abs there), all mins on DVE (bf16 2x), abs on ACT, no
  tensor_tensor_reduce (rejected by the runtime).
"""

import numpy as np

import concourse.bacc as bacc
import concourse.bass as bass  # noqa: F401
import concourse.tile as tile
from concourse import mybir

F32 = mybir.dt.float32
BF16 = mybir.dt.bfloat16
ALU = mybir.AluOpType
ACTF = mybir.ActivationFunctionType
AX = mybir.AxisListType

P = 128
N_CORES = 8
N_TOTAL = 262144
NC_N = N_TOTAL // N_CORES  # 32768 rows per core
G = 64                     # rows per partition per tile -> 4 tiles
NCHUNK = 2                 # phase-0 chunks

# accT slots per tile
E3S, SQA, SQB, L1S = 0, 1, 2, 3
NSLOT = 4


def build_nc(nc_n=NC_N, g=G):
    GA = nc_n // P
    ntiles = GA // g
    assert ntiles * P * g == nc_n

    nc = bacc.Bacc("TRN2", target_bir_lowering=False, debug=False)

    gt = nc.dram_tensor("gt", [nc_n, 9], F32, kind="ExternalInput").ap()
    pred = nc.dram_tensor("pred", [nc_n, 24], BF16, kind="ExternalInput").ap()
    ps = nc.dram_tensor("ps", [nc_n, 3], BF16, kind="ExternalInput").ap()
    out = nc.dram_tensor("out", [P, ntiles * NSLOT], F32, kind="ExternalOutput").ap()

    gt_r = gt.rearrange("(p g) f -> p g f", p=P)
    pred_r = pred.rearrange("(p g) f -> p g f", p=P)
    ps_r = ps.rearrange("(p g) f -> p g f", p=P)

    def Dpa_bc(Dpm, sl, j):
        return Dpm[:, sl, j : j + 1].broadcast_to((P, G, 8))

    with tile.TileContext(nc) as tc:
        with (
            tc.tile_pool(name="per", bufs=1) as per,   # persistent / phase-0
            tc.tile_pool(name="io", bufs=4) as io,
            tc.tile_pool(name="p0", bufs=2) as p0,
            tc.tile_pool(name="scr", bufs=2) as scr,
            tc.tile_pool(name="xe", bufs=2) as xe,     # cross-engine handoffs
        ):
            accT = per.tile([P, ntiles, NSLOT], F32)
            nc.vector.memset(accT, 0.0)

            # warm the ACT function tables before any data dependency
            warm = per.tile([P, 2], F32)
            nc.vector.memset(warm, 1.0)
            for fn in (ACTF.Square, ACTF.Sqrt):
                nc.scalar.activation(warm[:, 0:1], warm[:, 1:2], fn)

            # ================= phase 0: scaled axes + a2 scalars ==============
            sta = per.tile([P, GA, 3], F32)            # s cols (persistent, for L1)
            vta = per.tile([P, GA, 3, 3], BF16)        # v' = [c0 rx, c1 ry, cz rxy]
            Dpm = per.tile([P, GA, 2], F32)            # [A + C, A - C]
            GC = GA // NCHUNK

            def do_chunk_head(c):
                cs = slice(c * GC, (c + 1) * GC)
                stc = sta[:, cs]
                nc.sync.dma_start(out=stc, in_=gt_r[:, cs, 0:3])
                gtc = p0.tile([P, GC, 6], F32, tag="gtc")   # rx, ry (chunk scratch)
                nc.sync.dma_start(out=gtc, in_=gt_r[:, cs, 3:9])

                # |rx|, |ry| chain
                sqt = p0.tile([P, GC, 2, 3], F32, tag="sqt")
                gv = gtc[:, :, 3:9].rearrange("p g (v d) -> p g v d", d=3)
                nc.vector.tensor_mul(sqt, gv, gv)
                n2t = p0.tile([P, GC, 2], F32, tag="n2t")
                nc.vector.tensor_reduce(n2t, sqt, axis=AX.X, op=ALU.add)
                srt = p0.tile([P, GC, 2], F32, tag="srt")
                nc.scalar.activation(srt, n2t, ACTF.Sqrt)    # |r|
                ivt = p0.tile([P, GC, 2], F32, tag="ivt")
                nc.vector.reciprocal(ivt, srt)               # 1/|r|
                c01t = p0.tile([P, GC, 2], F32, tag="c01t")  # s0/|rx|, s1/|ry|
                nc.vector.tensor_mul(c01t, stc[:, :, 0:2], ivt)
                tzt = p0.tile([P, GC, 1], F32, tag="tzt")
                nc.gpsimd.tensor_mul(tzt, ivt[:, :, 0:1], ivt[:, :, 1:2])
                czt = p0.tile([P, GC, 1], F32, tag="czt")    # s2/(|rx||ry|)
                nc.gpsimd.tensor_mul(czt, stc[:, :, 2:3], tzt)

                # cross product (raw rx x ry) on GPSIMD
                rxet = p0.tile([P, GC, 5], F32, tag="rxet")
                ryet = p0.tile([P, GC, 5], F32, tag="ryet")
                nc.scalar.copy(rxet[:, :, 0:3], gtc[:, :, 0:3])
                nc.scalar.copy(rxet[:, :, 3:5], gtc[:, :, 0:2])
                nc.scalar.copy(ryet[:, :, 0:3], gtc[:, :, 3:6])
                nc.scalar.copy(ryet[:, :, 3:5], gtc[:, :, 3:5])
                m1t = p0.tile([P, GC, 3], F32, tag="m1t")
                m2t = p0.tile([P, GC, 3], F32, tag="m2t")
                crt = p0.tile([P, GC, 3], F32, tag="crt")
                nc.gpsimd.tensor_mul(m1t, rxet[:, :, 1:4], ryet[:, :, 2:5])
                nc.gpsimd.tensor_mul(m2t, rxet[:, :, 2:5], ryet[:, :, 1:4])
                nc.gpsimd.tensor_sub(crt, m1t, m2t)

                # scaled axes v' = 2*[v0,v1,v2], straight to bf16
                vtc = vta[:, cs]
                nc.vector.tensor_mul(
                    vtc[:, :, 0:2, :],
                    gtc.rearrange("p g (v d) -> p g v d", d=3),
                    c01t.unsqueeze(3).broadcast_to((P, GC, 2, 3)),
                )
                nc.gpsimd.tensor_mul(
                    vtc[:, :, 2, :], crt, czt.broadcast_to((P, GC, 3))
                )

            def do_chunk_tail(c):
                cs = slice(c * GC, (c + 1) * GC)
                vtc = vta[:, cs]
                # ---- a2 scalars: D+- = |v0 +- v1|^2 + |v2|^2  (vta = 2v)
                wpm = p0.tile([P, GC, 2, 3], BF16, tag="wpm")
                nc.vector.tensor_add(wpm[:, :, 0, :], vtc[:, :, 0, :], vtc[:, :, 1, :])
                nc.vector.tensor_sub(wpm[:, :, 1, :], vtc[:, :, 0, :], vtc[:, :, 1, :])
                sqw = p0.tile([P, GC, 2, 3], F32, tag="sqw")
                nc.scalar.activation(sqw, wpm, ACTF.Square, scale=0.5)  # |w/2|^2
                v2q = p0.tile([P, GC, 3], F32, tag="v2q")
                nc.scalar.activation(v2q, vtc[:, :, 2, :], ACTF.Square, scale=0.5)
                ds1 = p0.tile([P, GC, 2], F32, tag="ds1")
                nc.vector.tensor_add(ds1, sqw[:, :, :, 0], sqw[:, :, :, 1])
                ds2 = p0.tile([P, GC, 2], F32, tag="ds2")
                nc.vector.tensor_add(ds2, ds1, sqw[:, :, :, 2])
                v2s = p0.tile([P, GC, 1], F32, tag="v2s")
                nc.vector.tensor_reduce(v2s, v2q, axis=AX.X, op=ALU.add)
                nc.vector.tensor_add(
                    Dpm[:, cs], ds2, v2s.broadcast_to((P, GC, 2))
                )
                junkA = p0.tile([P, GC, 2], F32, tag="junkA")
                nc.scalar.activation(
                    junkA, Dpm[:, cs], ACTF.Identity, scale=0.5,
                    accum_out=accT[:, 2 * c, SQA : SQA + 1],
                )

            # ================= phase 1: pairwise chamfer, software-pipelined ==
            tiles = {}

            def s0(t):  # DMA + squares + products
                sl = slice(t * g, (t + 1) * g)
                d = tiles[t] = {"sl": sl}
                vt = vta[:, sl]
                bt = d["bt"] = io.tile([P, g, 8, 3], BF16, tag="pred")
                pst = d["pst"] = io.tile([P, g, 3], BF16, tag="ps")
                nc.sync.dma_start(
                    out=bt, in_=pred_r[:, sl].rearrange("p g (q d) -> p g q d", d=3)
                )
                nc.sync.dma_start(out=pst, in_=ps_r[:, sl])
                sqbtT = d["sqbtT"] = xe.tile([P, g, 3, 8], BF16, tag="sqbtT")
                nc.scalar.activation(
                    sqbtT.transpose([0, 1, 3, 2]), bt, ACTF.Square,
                    accum_out=accT[:, t, SQB : SQB + 1],
                )
                h3 = d["h3"] = scr.tile([P, g, 3, 8, 3], BF16, tag="h3")
                nc.vector.tensor_tensor(
                    h3,
                    vt.unsqueeze(3).broadcast_to((P, g, 3, 8, 3)),
                    bt.unsqueeze(2).broadcast_to((P, g, 3, 8, 3)),
                    op=ALU.mult,
                )

            def s1(t):  # d-sums + b2
                d = tiles[t]
                h3 = d["h3"]
                hs1 = d["hs1"] = xe.tile([P, g, 3, 8], BF16, tag="hs1")
                nc.gpsimd.tensor_add(hs1, h3[:, :, :, :, 0], h3[:, :, :, :, 1])
                b2s = d["b2s"] = scr.tile([P, g, 8], BF16, tag="b2s")
                nc.vector.tensor_add(b2s, d["sqbtT"][:, :, 0, :], d["sqbtT"][:, :, 1, :])

            def s2(t):
                d = tiles[t]
                ht = d["ht"] = scr.tile([P, g, 3, 8], BF16, tag="ht")
                nc.vector.tensor_add(ht, d["hs1"], d["h3"][:, :, :, :, 2])
                b2b = d["b2b"] = scr.tile([P, g, 8], BF16, tag="b2b")
                nc.vector.tensor_add(b2b, d["b2s"], d["sqbtT"][:, :, 2, :])

            def s3(t):  # S-combos
                d = tiles[t]
                ht = d["ht"]
                w01 = d["w01"] = scr.tile([P, g, 2, 8], BF16, tag="w01")
                nc.vector.tensor_add(w01[:, :, 0, :], ht[:, :, 0, :], ht[:, :, 1, :])
                nc.vector.tensor_sub(w01[:, :, 1, :], ht[:, :, 0, :], ht[:, :, 1, :])
                tpm = d["tpm"] = scr.tile([P, g, 2, 8], BF16, tag="tpm")
                nc.vector.tensor_sub(tpm[:, :, 0, :], d["b2b"], ht[:, :, 2, :])
                nc.vector.tensor_add(tpm[:, :, 1, :], d["b2b"], ht[:, :, 2, :])
                aw = d["aw"] = scr.tile([P, g, 2, 8], BF16, tag="aw")
                nc.vector.tensor_scalar(aw, w01, 0.0, None, op0=ALU.abs_max)
                ah2 = d["ah2"] = scr.tile([P, g, 8], BF16, tag="ah2")
                nc.vector.tensor_scalar(ah2, ht[:, :, 2, :], 0.0, None, op0=ALU.abs_max)

            def s4(t):  # em/ep + dist2 subs
                d = tiles[t]
                sl = d["sl"]
                w01, tpm = d["w01"], d["tpm"]
                eeb = d["eeb"] = scr.tile([P, g, 8, 8], BF16, tag="eeb")
                tm2 = tpm[:, :, 0:1, :].broadcast_to((P, g, 2, 8))
                tp2 = tpm[:, :, 1:2, :].broadcast_to((P, g, 2, 8))
                nc.vector.tensor_sub(eeb[:, :, 0:2, :], tm2, w01)
                nc.vector.tensor_sub(eeb[:, :, 2:4, :], tp2, w01)
                nc.vector.tensor_add(eeb[:, :, 4:6, :], tp2, w01)
                nc.vector.tensor_add(eeb[:, :, 6:8, :], tm2, w01)
                t0 = d["t0"] = xe.tile([P, g, 8], F32, tag="t0")
                nc.gpsimd.tensor_sub(t0, Dpa_bc(Dpm, sl, 0), d["aw"][:, :, 0, :])
                t1 = d["t1"] = xe.tile([P, g, 8], F32, tag="t1")
                nc.gpsimd.tensor_sub(t1, Dpa_bc(Dpm, sl, 1), d["aw"][:, :, 1, :])
                l1d = d["l1d"] = xe.tile([P, g, 3], F32, tag="l1d")
                nc.gpsimd.tensor_sub(l1d, d["pst"], sta[:, sl])

            def s5(t):  # tree L1 + dist2 min
                d = tiles[t]
                e1 = d["e1"] = scr.tile([P, g, 8, 4], BF16, tag="e1")
                nc.vector.tensor_tensor(
                    e1, d["eeb"][:, :, :, 0:4], d["eeb"][:, :, :, 4:8], op=ALU.min
                )
                m01 = d["m01"] = xe.tile([P, g, 8], F32, tag="m01")
                nc.gpsimd.tensor_tensor(m01, d["t0"], d["t1"], op=ALU.min)
                junk3 = scr.tile([P, g, 3], F32, tag="junk3")
                nc.scalar.activation(
                    junk3, d["l1d"], ACTF.Abs, accum_out=accT[:, t, L1S : L1S + 1]
                )

            def s6(t):  # tree L2 + fused accumulating finals
                d = tiles[t]
                e1 = d["e1"]
                e2 = d["e2"] = scr.tile([P, g, 8, 2], BF16, tag="e2")
                nc.vector.tensor_tensor(
                    e2, e1[:, :, :, 0:2], e1[:, :, :, 2:4], op=ALU.min
                )
                d2 = scr.tile([P, g, 8], BF16, tag="d2")
                nc.gpsimd.tensor_sub(d2, d["m01"], d["ah2"])
                junkd = scr.tile([P, g, 8], BF16, tag="junkd")
                nc.scalar.activation(
                    junkd, d2, ACTF.Identity, accum_out=accT[:, t, D2S : D2S + 1]
                )

            def s7(t):
                d = tiles[t]
                e2 = d["e2"]
                est = d["est"]
                nc.vector.tensor_tensor(
                    est[:, :, 0:8], e2[:, :, :, 0], e2[:, :, :, 1], op=ALU.min
                )
                junke = scr.tile([P, g, 16], BF16, tag="junke")
                nc.scalar.activation(
                    junke, est, ACTF.Identity, accum_out=accT[:, t, E3S : E3S + 1]
                )
                nc.sync.dma_start(
                    out=out[:, t * NSLOT : (t + 1) * NSLOT], in_=accT[:, t]
                )

            stages = [s0, s1, s2, s3, s4, s5, s6, s7]

            for c in range(NCHUNK):
                do_chunk_head(c)
            for c in range(NCHUNK):
                do_chunk_tail(c)
            # wavefront emission: stage s of tile t at wave s + t
            for w in range(ntiles + len(stages) - 1):
                for t in range(ntiles):
                    s = w - t
                    if 0 <= s < len(stages):
                        stages[s](t)


    nc.compile()
    return nc


_CACHE = {}


def _get_nc():
    if "nc" not in _CACHE:
        _CACHE["nc"] = build_nc()
    return _CACHE["nc"]


def make_in_maps(pred_pts, pred_size, gt_box):
    """Host-side shard + layout prep (bf16 casts, gt column slice)."""
    import ml_dtypes

    pred_pts = np.asarray(pred_pts, dtype=np.float32)
    pred_size = np.asarray(pred_size, dtype=np.float32)
    gt_box = np.asarray(gt_box, dtype=np.float32)

    N = pred_pts.shape[0]
    assert N == N_TOTAL, f"expected {N_TOTAL} rows, got {N}"
    gt9 = np.ascontiguousarray(gt_box.reshape(N, 12)[:, 3:12])
    pred = np.ascontiguousarray(
        pred_pts.reshape(N, 24).astype(ml_dtypes.bfloat16)
    )
    psb = np.ascontiguousarray(pred_size.astype(ml_dtypes.bfloat16))
    return [
        {
            "gt": gt9[i * NC_N : (i + 1) * NC_N],
            "pred": pred[i * NC_N : (i + 1) * NC_N],
            "ps": psb[i * NC_N : (i + 1) * NC_N],
        }
        for i in range(N_CORES)
    ]


def combine_partials(outs):
    """outs: list of (P, ntiles*NSLOT) arrays -> (cd_sum, l1_sum) float64."""
    tot = np.zeros(NSLOT, dtype=np.float64)
    for o in outs:
        o = o.astype(np.float64).reshape(P, -1, NSLOT)
        tot += o.sum(axis=(0, 1))
    cd_sum = tot[E3S] + 8.0 * tot[SQA] + tot[SQB]
    return cd_sum, tot[L1S]


def kernel(pred_pts, pred_size, gt_box):
    from concourse.bass_utils import run_bass_kernel_spmd

    in_maps = make_in_maps(pred_pts, pred_size, gt_box)
    res = run_bass_kernel_spmd(_get_nc(), in_maps, core_ids=list(range(N_CORES)))
    cd_sum, l1_sum = combine_partials([r["out"] for r in res.results])
    N = N_TOTAL
    cd = cd_sum / (N * 8)
    l1 = l1_sum / (N * 3)
    return np.array([cd, l1], dtype=np.float32)
